# revision 5
# baseline (speedup 1.0000x reference)
"""Trainium2 Bass kernel for nn_CharDecoder (LSTM char decoder).

Strategy (data-parallel over batch, 8 cores x 256 batch each):
  - Host folds the embedding + input projection into a [96, 2048] lookup
    table:  table[v] = emb[v] @ W_ih.T + b_ih + b_hh, gathers the per-step
    input-side gate pre-activations x_gates[l] = table[ids[l]] (bf16), and
    ships them; the device streams them from HBM and adds them on the
    vector engine (which has large slack) instead of burning PE cycles.
  - Everything on device lives in [feature, batch] (transposed) layout so
    the recurrence h_{t+1} = f(W h_t + ...) needs no transposes at all.
  - Matmuls run in float32r (full PE rate for moving free dim >= 256,
    fp32 storage, relaxed-precision multiply, fp32 PSUM accumulate).
  - Emission is chunk-major (h-chunk j outer, gate inner) with per-chunk
    elementwise so h chunks are ready just in time for the next step's
    k-tile matmuls; the projection of step l-1 is emitted in the middle
    of step l's matmul stream (h is parity double-buffered for this).
"""

import sys

sys.path.insert(0, "/opt/trn_rl_repo")

import ml_dtypes
import numpy as np

import concourse.mybir as mybir  # noqa: E402
import concourse.tile as tile  # noqa: E402
from concourse import bacc  # noqa: E402
from concourse import bass_utils  # noqa: E402

L, B, H, E, V = 64, 2048, 512, 50, 96
NCORES = 8
BC = B // NCORES  # batch per core = 256
KH = H // 128  # 4 k-tiles over hidden dim
MT = (4 * H) // 128  # 16 m-tiles over gate dim
P = 128

F32 = mybir.dt.float32
F32R = mybir.dt.float32r
BF16 = mybir.dt.bfloat16
AF = mybir.ActivationFunctionType
ALU = mybir.AluOpType
BF16_NP = ml_dtypes.bfloat16


def build_nc(n_steps=L):
    """Build the Bass program (single-core SPMD; same NEFF on all cores)."""
    nc = bacc.Bacc("TRN2", target_bir_lowering=False, debug=False)

    waug_d = nc.dram_tensor("waug", (P, KH, 4 * H), F32R, kind="ExternalInput").ap()
    xg_d = nc.dram_tensor("xg", (P, n_steps, MT, BC), BF16, kind="ExternalInput").ap()
    h0_d = nc.dram_tensor("h0t", (P, KH * BC), F32R, kind="ExternalInput").ap()
    c0_d = nc.dram_tensor("c0t", (P, KH * BC), F32, kind="ExternalInput").ap()
    wo_d = nc.dram_tensor("woutt", (P, KH, V), F32R, kind="ExternalInput").ap()
    bo_d = nc.dram_tensor("bout", (V, 1), F32, kind="ExternalInput").ap()

    sc_d = nc.dram_tensor("scores", (n_steps, V, BC), F32, kind="ExternalOutput").ap()
    hN_d = nc.dram_tensor("hN", (P, KH * BC), F32R, kind="ExternalOutput").ap()
    cN_d = nc.dram_tensor("cN", (P, KH * BC), F32, kind="ExternalOutput").ap()

    with tile.TileContext(nc) as tc:
        _emit(tc, n_steps, waug_d, xg_d, h0_d, c0_d, wo_d, bo_d, sc_d, hN_d, cN_d)
    nc.compile()
    return nc


def _emit(tc, n_steps, waug_d, xg_d, h0_d, c0_d, wo_d, bo_d, sc_d, hN_d, cN_d):
    nc = tc.nc
    with (
        tc.tile_pool(name="const", bufs=1) as const,
        tc.tile_pool(name="xgp", bufs=3) as xgp,
        tc.tile_pool(name="acts", bufs=2) as acts,
        tc.tile_pool(name="tmps", bufs=3) as tmps,
        tc.tile_pool(name="scst", bufs=3) as scst,
        tc.tile_pool(name="psg", bufs=7, space="PSUM") as psg,
        tc.tile_pool(name="psp", bufs=1, space="PSUM") as psp,
    ):
        # --- persistent state / weights in SBUF ---
        w_sb = const.tile([P, KH, 4 * H], F32R)
        nc.sync.dma_start(w_sb[:], waug_d)
        wo_sb = const.tile([P, KH, V], F32R)
        nc.sync.dma_start(wo_sb[:], wo_d)
        bo_sb = const.tile([V, 1], F32)
        nc.sync.dma_start(bo_sb[:], bo_d)
        # h double-buffered by step parity so the deferred projection of
        # step l-1 can read it while step l's chunks overwrite the other.
        h_sb = const.tile([P, 2, KH * BC], F32R)
        nc.sync.dma_start(h_sb[:, 1], h0_d)
        c_sb = const.tile([P, KH * BC], F32)
        nc.sync.dma_start(c_sb[:], c0_d)

        gate_func = [AF.Sigmoid, AF.Sigmoid, AF.Tanh, AF.Sigmoid]

        def emit_proj(l):
            # scoresT[l] = W_out @ h(l) + b_out   -> [96, 256]
            pp = psp.tile([P, BC], F32, name="pp", tag="pp")
            for k in range(KH):
                nc.tensor.matmul(
                    pp[:V],
                    wo_sb[:, k, :],
                    h_sb[:, l % 2, k * BC : (k + 1) * BC],
                    start=(k == 0),
                    stop=(k == KH - 1),
                )
            sc = scst.tile([V, BC], F32, tag="sc")
            nc.scalar.activation(sc[:], pp[:V], AF.Identity, bias=bo_sb[:])
            nc.sync.dma_start(sc_d[l], sc[:])

        for l in range(n_steps):
            pr, cu = (l + 1) % 2, l % 2  # h parity: read pr, write cu
            # streamed input-side gate pre-activations for this step
            xg_sb = xgp.tile([P, MT, BC], BF16, tag="xg")
            nc.sync.dma_start(xg_sb[:], xg_d[:, l])
            # gate activations, [128, 4*256]: chunk j in cols j*BC:(j+1)*BC
            s_g = [
                acts.tile([P, KH * BC], F32, name=f"s{g}", tag=f"s{g}")
                for g in range(4)
            ]

            # emission order is chunk-major: tile t = j*4 + gi covers
            # h-chunk j of gate gi; weight m-tile index is gi*4 + j.
            for t in range(MT):
                j, gi = divmod(t, 4)
                m = gi * 4 + j
                ps = psg.tile([P, BC], F32, name="ps", tag="ps")
                for k in range(KH):
                    nc.tensor.matmul(
                        ps[:],
                        w_sb[:, k, m * P : (m + 1) * P],
                        h_sb[:, pr, k * BC : (k + 1) * BC],
                        start=(k == 0),
                        stop=(k == KH - 1),
                    )
                pre = tmps.tile([P, BC], F32, tag="pre")
                nc.vector.tensor_tensor(pre[:], ps[:], xg_sb[:, m, :], ALU.add)
                nc.scalar.activation(
                    s_g[gi][:, j * BC : (j + 1) * BC], pre[:], gate_func[gi]
                )
                if t == 3 and l > 0:
                    emit_proj(l - 1)  # deferred: h(l-1) is long done
                if gi == 3:
                    # chunk j complete: c_j, h_j elementwise ([128, 256])
                    cj = slice(j * BC, (j + 1) * BC)
                    t2 = tmps.tile([P, BC], F32, tag="t2")
                    nc.vector.tensor_tensor(
                        t2[:], s_g[0][:, cj], s_g[2][:, cj], ALU.mult
                    )
                    t1 = tmps.tile([P, BC], F32, tag="t1")
                    nc.vector.tensor_tensor(t1[:], s_g[1][:, cj], c_sb[:, cj], ALU.mult)
                    nc.vector.tensor_tensor(c_sb[:, cj], t1[:], t2[:], ALU.add)
                    tct = tmps.tile([P, BC], F32, tag="tct")
                    nc.scalar.activation(tct[:], c_sb[:, cj], AF.Tanh)
                    nc.vector.tensor_tensor(
                        h_sb[:, cu, cj], s_g[3][:, cj], tct[:], ALU.mult
                    )

        emit_proj(n_steps - 1)
        nc.sync.dma_start(hN_d, h_sb[:, (n_steps - 1) % 2])
        nc.sync.dma_start(cN_d, c_sb[:])


# ---------------------------------------------------------------------------
# host-side pre/post processing
# ---------------------------------------------------------------------------


def make_in_maps(inputs, n_steps=L):
    ids = np.asarray(inputs["input_ids"]).astype(np.int64)[:n_steps]  # (l, B)
    emb = np.asarray(inputs["emb"], dtype=np.float32)
    W_ih = np.asarray(inputs["W_ih"], dtype=np.float32)
    W_hh = np.asarray(inputs["W_hh"], dtype=np.float32)
    b_ih = np.asarray(inputs["b_ih"], dtype=np.float32)
    b_hh = np.asarray(inputs["b_hh"], dtype=np.float32)
    W_out = np.asarray(inputs["W_out"], dtype=np.float32)
    b_out = np.asarray(inputs["b_out"], dtype=np.float32)
    h0 = np.asarray(inputs["h0"], dtype=np.float32)[0]  # (B, H)
    c0 = np.asarray(inputs["c0"], dtype=np.float32)[0]

    table = emb @ W_ih.T + b_ih + b_hh  # (V, 4H)
    waug = np.ascontiguousarray(
        W_hh.T.reshape(KH, P, 4 * H).transpose(1, 0, 2)
    )  # (128, KH, 4H)
    wout = np.ascontiguousarray(W_out.T.reshape(KH, P, V).transpose(1, 0, 2))
    bout = np.ascontiguousarray(b_out.reshape(V, 1))
    # table arranged for the device x_gates layout: [p, v, m] bf16
    t_pm = np.ascontiguousarray(
        table.reshape(V, MT, P).transpose(2, 0, 1).astype(BF16_NP)
    )  # (128, V, MT)

    def pack_state(x):  # (BC, H) -> (128, KH*BC) transposed-chunked
        xT = x.T.reshape(KH, P, BC).transpose(1, 0, 2)
        return np.ascontiguousarray(xT.reshape(P, KH * BC))

    in_maps = []
    for c in range(NCORES):
        sl = slice(c * BC, (c + 1) * BC)
        idc = ids[:, sl]  # (l, BC)
        # xg[p, l, m, b] = table[ids[l, b], m*128+p]
        xg = t_pm[:, idc, :]  # (128, l, BC, MT)
        xg = np.ascontiguousarray(xg.transpose(0, 1, 3, 2))  # (128, l, MT, BC)
        in_maps.append(
            {
                "waug": waug,
                "xg": xg,
                "h0t": pack_state(h0[sl]),
                "c0t": pack_state(c0[sl]),
                "woutt": wout,
                "bout": bout,
            }
        )
    return in_maps


def unpack_state(x):  # (128, KH*BC) -> (BC, H)
    return x.reshape(P, KH, BC).transpose(1, 0, 2).reshape(H, BC).T


def assemble_outputs(results, n_steps=L):
    scores = np.concatenate(
        [r["scores"].transpose(0, 2, 1) for r in results], axis=1
    )  # (l, B, V)
    hN = np.concatenate([unpack_state(r["hN"]) for r in results], axis=0)[None]
    cN = np.concatenate([unpack_state(r["cN"]) for r in results], axis=0)[None]
    return scores, hN, cN


_NC_CACHE = {}


def run(inputs, n_steps=L, trace=False):
    if n_steps not in _NC_CACHE:
        _NC_CACHE[n_steps] = build_nc(n_steps)
    nc = _NC_CACHE[n_steps]
    in_maps = make_in_maps(inputs, n_steps)
    res = bass_utils.run_bass_kernel_spmd(
        nc, in_maps, core_ids=list(range(NCORES)), trace=trace
    )
    out = assemble_outputs(res.results, n_steps)
    return out, res


def kernel(**inputs):
    out, _ = run(inputs, L, trace=False)
    return out


# revision 12
# speedup vs baseline: 2.9768x; 2.9768x over previous
"""Trainium2 Bass kernel for nn_CharDecoder (LSTM char decoder).

Strategy (data-parallel over batch, 8 cores x 256 batch each):
  - Host folds the embedding + input projection into a [96, 2048] lookup
    table:  table[v] = emb[v] @ W_ih.T + b_ih + b_hh.  The per-step input
    gate contribution x_gates[l] = table[ids[l]] is realized on-device as a
    one-hot matmul accumulated into the same PSUM tile as the recurrent
    matmul (a 5th K-tile of an augmented state).
  - Everything on device lives in [feature, batch] (transposed) layout so
    the recurrence h_{t+1} = f(W h_t + ...) needs no transposes at all.
  - Matmuls run in float32r (full PE rate for moving free dim >= 256,
    fp32 storage, relaxed-precision multiply, fp32 PSUM accumulate).
"""

import sys

sys.path.insert(0, "/opt/trn_rl_repo")

import numpy as np

import concourse.mybir as mybir  # noqa: E402
import concourse.tile as tile  # noqa: E402
from concourse import bacc  # noqa: E402
from concourse import bass_utils  # noqa: E402

L, B, H, E, V = 64, 2048, 512, 50, 96
NCORES = 8
BC = B // NCORES  # batch per core = 256
KH = H // 128  # 4 k-tiles over hidden dim
MT = (4 * H) // 128  # 16 m-tiles over gate dim
P = 128

F32 = mybir.dt.float32
F32R = mybir.dt.float32r
AF = mybir.ActivationFunctionType
ALU = mybir.AluOpType


def _r(ap):
    return ap.bitcast(F32R)


def build_nc(n_steps=L):
    """Build the Bass program (single-core SPMD; same NEFF on all cores)."""
    nc = bacc.Bacc("TRN2", target_bir_lowering=False, debug=False)

    waug_d = nc.dram_tensor("waug", (P, 5, 4 * H), F32R, kind="ExternalInput").ap()
    oh_d = nc.dram_tensor("onehot", (P, n_steps, BC), F32R, kind="ExternalInput").ap()
    h0_d = nc.dram_tensor("h0t", (P, KH * BC), F32R, kind="ExternalInput").ap()
    c0_d = nc.dram_tensor("c0t", (P, KH * BC), F32, kind="ExternalInput").ap()
    wo_d = nc.dram_tensor("woutt", (P, KH, V), F32R, kind="ExternalInput").ap()
    bo_d = nc.dram_tensor("bout", (V, 1), F32, kind="ExternalInput").ap()

    sc_d = nc.dram_tensor("scores", (n_steps, V, BC), F32, kind="ExternalOutput").ap()
    hN_d = nc.dram_tensor("hN", (P, KH * BC), F32R, kind="ExternalOutput").ap()
    cN_d = nc.dram_tensor("cN", (P, KH * BC), F32, kind="ExternalOutput").ap()

    with tile.TileContext(nc) as tc:
        _emit(tc, n_steps, waug_d, oh_d, h0_d, c0_d, wo_d, bo_d, sc_d, hN_d, cN_d)
    nc.compile()
    return nc


def _emit(tc, n_steps, waug_d, oh_d, h0_d, c0_d, wo_d, bo_d, sc_d, hN_d, cN_d):
    nc = tc.nc
    with (
        tc.tile_pool(name="const", bufs=1) as const,
        tc.tile_pool(name="acts", bufs=2) as acts,
        tc.tile_pool(name="tmps", bufs=3) as tmps,
        tc.tile_pool(name="scst", bufs=3) as scst,
        tc.tile_pool(name="psg", bufs=7, space="PSUM") as psg,
        tc.tile_pool(name="psp", bufs=1, space="PSUM") as psp,
    ):
        # --- persistent state / weights in SBUF ---
        w_sb = const.tile([P, 5, 4 * H], F32R)
        nc.sync.dma_start(w_sb[:], waug_d)
        oh_sb = const.tile([P, n_steps, BC], F32R)
        nc.sync.dma_start(oh_sb[:], oh_d)
        wo_sb = const.tile([P, KH, V], F32R)
        nc.sync.dma_start(wo_sb[:], wo_d)
        bo_sb = const.tile([V, 1], F32)
        nc.sync.dma_start(bo_sb[:], bo_d)
        # h double-buffered by step parity so the deferred projection of
        # step l-1 can read it while step l's chunks overwrite the other.
        h_sb = const.tile([P, 2, KH * BC], F32R)
        nc.sync.dma_start(h_sb[:, 1], h0_d)
        c_sb = const.tile([P, KH * BC], F32)
        nc.sync.dma_start(c_sb[:], c0_d)

        gate_func = [AF.Sigmoid, AF.Sigmoid, AF.Tanh, AF.Sigmoid]
        WARM = 7  # onehot matmuls issued ahead (== psg bufs)

        def emit_proj(l):
            # scoresT[l] = W_out @ h(l) + b_out   -> [96, 256]
            pp = psp.tile([P, 2 * BC], F32, name="pp", tag="pp")
            for k in range(KH):
                nc.tensor.matmul(
                    pp[:V, :BC],
                    wo_sb[:, k, :],
                    h_sb[:, l % 2, k * BC : (k + 1) * BC],
                    start=(k == 0),
                    stop=(k == KH - 1),
                )
            sc = scst.tile([V, BC], F32, tag="sc")
            nc.scalar.activation(sc[:], pp[:V, :BC], AF.Identity, bias=bo_sb[:])
            nc.sync.dma_start(sc_d[l], sc[:])

        def emit_proj_pair(a):
            # scoresT[a:a+2] = W_out @ [h(a) | h(a+1)] + b_out; a is even so
            # h parity 0 holds step a... (parity p holds step a+p).
            pp = psp.tile([P, 2 * BC], F32, name="pp", tag="pp")
            for k in range(KH):
                nc.tensor.matmul(
                    pp[:V],
                    wo_sb[:, k, :],
                    h_sb[:, :, k * BC : (k + 1) * BC],
                    start=(k == 0),
                    stop=(k == KH - 1),
                )
            sc2 = scst.tile([V, 2 * BC], F32, tag="sc2")
            nc.scalar.activation(sc2[:], pp[:V], AF.Identity, bias=bo_sb[:])
            nc.sync.dma_start(
                sc_d[a : a + 2].rearrange("l v b -> v l b"),
                sc2[:].rearrange("v (l b) -> v l b", l=2),
            )

        for l in range(n_steps):
            pr, cu = (l + 1) % 2, l % 2  # h parity: read pr, write cu
            # gate activations, [128, 4*256]: chunk j in cols j*BC:(j+1)*BC
            s_g = [
                acts.tile([P, KH * BC], F32, name=f"s{g}", tag=f"s{g}")
                for g in range(4)
            ]

            # emission order is chunk-major: tile t = j*4 + gi covers
            # h-chunk j of gate gi; weight m-tile index is gi*4 + j.
            ps_t = [None] * MT

            def start_tile(t):
                j, gi = divmod(t, 4)
                m = gi * 4 + j
                ps_t[t] = psg.tile([P, BC], F32, name="ps", tag="ps")
                nc.tensor.matmul(
                    ps_t[t],
                    w_sb[:, 4, m * P : (m + 1) * P],
                    oh_sb[:, l, :],
                    start=True,
                    stop=False,
                )

            for t in range(WARM):
                start_tile(t)
            for t in range(MT):
                j, gi = divmod(t, 4)
                m = gi * 4 + j
                for k in range(KH):
                    nc.tensor.matmul(
                        ps_t[t],
                        w_sb[:, k, m * P : (m + 1) * P],
                        h_sb[:, pr, k * BC : (k + 1) * BC],
                        start=False,
                        stop=(k == KH - 1),
                    )
                nc.scalar.activation(
                    s_g[gi][:, j * BC : (j + 1) * BC], ps_t[t], gate_func[gi]
                )
                if t + WARM < MT:
                    start_tile(t + WARM)
                if t == 3 and l >= 2 and l % 2 == 0:
                    emit_proj_pair(l - 2)  # deferred: h(l-2), h(l-1) long done
                if gi == 3:
                    # chunk j complete: c_j, h_j elementwise ([128, 256])
                    cj = slice(j * BC, (j + 1) * BC)
                    t2 = tmps.tile([P, BC], F32, tag="t2")
                    nc.vector.tensor_tensor(
                        t2[:], s_g[0][:, cj], s_g[2][:, cj], ALU.mult
                    )
                    t1 = tmps.tile([P, BC], F32, tag="t1")
                    nc.vector.tensor_tensor(t1[:], s_g[1][:, cj], c_sb[:, cj], ALU.mult)
                    nc.vector.tensor_tensor(c_sb[:, cj], t1[:], t2[:], ALU.add)
                    tct = tmps.tile([P, BC], F32, tag="tct")
                    nc.scalar.activation(tct[:], c_sb[:, cj], AF.Tanh)
                    nc.vector.tensor_tensor(
                        h_sb[:, cu, cj], s_g[3][:, cj], tct[:], ALU.mult
                    )

        if n_steps % 2 == 0:
            emit_proj_pair(n_steps - 2)
        else:
            # in-loop pairs already covered up to step n_steps-2
            emit_proj(n_steps - 1)
        nc.sync.dma_start(hN_d, h_sb[:, (n_steps - 1) % 2])
        nc.sync.dma_start(cN_d, c_sb[:])


# ---------------------------------------------------------------------------
# host-side pre/post processing
# ---------------------------------------------------------------------------


def make_in_maps(inputs, n_steps=L):
    ids = np.asarray(inputs["input_ids"]).astype(np.int64)[:n_steps]  # (l, B)
    emb = np.asarray(inputs["emb"], dtype=np.float32)
    W_ih = np.asarray(inputs["W_ih"], dtype=np.float32)
    W_hh = np.asarray(inputs["W_hh"], dtype=np.float32)
    b_ih = np.asarray(inputs["b_ih"], dtype=np.float32)
    b_hh = np.asarray(inputs["b_hh"], dtype=np.float32)
    W_out = np.asarray(inputs["W_out"], dtype=np.float32)
    b_out = np.asarray(inputs["b_out"], dtype=np.float32)
    h0 = np.asarray(inputs["h0"], dtype=np.float32)[0]  # (B, H)
    c0 = np.asarray(inputs["c0"], dtype=np.float32)[0]

    table = emb @ W_ih.T + b_ih + b_hh  # (V, 4H)
    waug = np.zeros((P, 5, 4 * H), np.float32)
    waug[:, :4, :] = W_hh.T.reshape(KH, P, 4 * H).transpose(1, 0, 2)
    waug[:V, 4, :] = table
    wout = np.ascontiguousarray(W_out.T.reshape(KH, P, V).transpose(1, 0, 2))
    bout = np.ascontiguousarray(b_out.reshape(V, 1))

    def pack_state(x):  # (BC, H) -> (128, KH*BC) transposed-chunked
        xT = x.T.reshape(KH, P, BC).transpose(1, 0, 2)
        return np.ascontiguousarray(xT.reshape(P, KH * BC))

    in_maps = []
    for c in range(NCORES):
        sl = slice(c * BC, (c + 1) * BC)
        idc = ids[:, sl]  # (l, BC)
        oh = (idc[:, None, :] == np.arange(P)[None, :, None]).astype(np.float32)
        oh = np.ascontiguousarray(oh.transpose(1, 0, 2))  # (128, l, BC)
        in_maps.append(
            {
                "waug": waug,
                "onehot": oh,
                "h0t": pack_state(h0[sl]),
                "c0t": pack_state(c0[sl]),
                "woutt": wout,
                "bout": bout,
            }
        )
    return in_maps


def unpack_state(x):  # (128, KH*BC) -> (BC, H)
    return x.reshape(P, KH, BC).transpose(1, 0, 2).reshape(H, BC).T


def assemble_outputs(results, n_steps=L):
    scores = np.concatenate(
        [r["scores"].transpose(0, 2, 1) for r in results], axis=1
    )  # (l, B, V)
    hN = np.concatenate([unpack_state(r["hN"]) for r in results], axis=0)[None]
    cN = np.concatenate([unpack_state(r["cN"]) for r in results], axis=0)[None]
    return scores, hN, cN


_NC_CACHE = {}


def run(inputs, n_steps=L, trace=False):
    if n_steps not in _NC_CACHE:
        _NC_CACHE[n_steps] = build_nc(n_steps)
    nc = _NC_CACHE[n_steps]
    in_maps = make_in_maps(inputs, n_steps)
    res = bass_utils.run_bass_kernel_spmd(
        nc, in_maps, core_ids=list(range(NCORES)), trace=trace
    )
    out = assemble_outputs(res.results, n_steps)
    return out, res


def kernel(**inputs):
    out, _ = run(inputs, L, trace=False)
    return out
